# revision 1
# baseline (speedup 1.0000x reference)
"""DeepSeek-V3 TopK router kernel for Trainium2 (8 NeuronCores, data-parallel).

Routing math per token (256 experts, 8 groups of 32):
  s = sigmoid(logits)                      [exact fp32: 1/(1+exp(-x))]
  biased = s + correction_bias
  group_score(g) = top2sum(biased[g])
  keep top-4 groups (threshold at 4th-largest group score)
  masked = biased * group_mask             [masked-out -> 0.0, as reference]
  top-8 experts of masked -> indices (desc order, exact)
  weights = s at those indices, normalized, *2.5   [2e-2 tolerance]

Selection math is bitwise-identical to the jax-on-neuron reference
(ACT Exp + exact +1.0 + DVE exact reciprocal + exact adds/compares,
consume-matching max_index for jax top_k tie order).

Weight extraction uses a rank-mark scatter: gpsimd.local_scatter writes
mark[k] = 8192 - 512k (bf16-exact) at expert eidx[k]; q = s + marks puts
the 8 winners at 4608..8192+u so one max8(q) returns them in slot order;
u = q - mark recovers sigmoid to ~2^-12 abs (weights tolerate 2e-2,
winner sigmoids are >= 0.49 on this distribution).
"""

import sys

for p in ("/opt/trn_rl_repo", "/opt/trn_rl_repo/concourse"):
    if p not in sys.path:
        sys.path.insert(0, p)

import numpy as np

N_TOKENS = 131072
N_EXPERTS = 256
N_GROUP = 8
GROUP_SIZE = 32
TOP_K = 8
N_CORES = 8
TOK_PER_CORE = N_TOKENS // N_CORES  # 16384
P = 128  # partitions / tokens per tile
B = 4    # tiles per batch
E4 = N_EXPERTS * B  # 1024

MARKS = [8192.0 - 512.0 * k for k in range(TOP_K)]  # bf16-exact rank marks

# If False, max_index searches the UNMASKED biased array (saves a Pool pass).
# Safe iff no winner value duplicates at an earlier position in a masked-out
# group anywhere in the dataset -- verified empirically on the fixed key=0 data.
MASKED_SEARCH = True

_COMPILED = {}


def _consts_row() -> np.ndarray:
    """[0:32]=marks per (tile-slot, rank) f32; [32:64]=index offsets 256*b."""
    row = np.zeros((1, 64), dtype=np.float32)
    row[0, 0:32] = np.tile(np.array(MARKS, dtype=np.float32), B)
    row[0, 32:64] = np.repeat(np.arange(B, dtype=np.float32) * N_EXPERTS, TOP_K)
    return row


def _build(tokens_per_core: int):
    import concourse.bass as bass  # noqa: F401
    import concourse.mybir as mybir
    import concourse.tile as tile
    from concourse import bacc

    f32 = mybir.dt.float32
    bf16 = mybir.dt.bfloat16
    i16 = mybir.dt.int16
    u16 = mybir.dt.uint16
    u32 = mybir.dt.uint32
    Alu = mybir.AluOpType
    Act = mybir.ActivationFunctionType
    X = mybir.AxisListType.X

    n_tiles = tokens_per_core // P
    assert tokens_per_core % (P * B) == 0
    n_batches = n_tiles // B
    BTOK = P * B  # tokens per batch

    nc = bacc.Bacc(None, target_bir_lowering=False)
    x = nc.dram_tensor("x", [tokens_per_core, N_EXPERTS], f32, kind="ExternalInput")
    cb = nc.dram_tensor("cb", [1, N_EXPERTS], f32, kind="ExternalInput")
    cst = nc.dram_tensor("cst", [1, 64], f32, kind="ExternalInput")
    ow = nc.dram_tensor("ow", [tokens_per_core, TOP_K], f32, kind="ExternalOutput")
    oi = nc.dram_tensor("oi", [tokens_per_core, TOP_K], u16, kind="ExternalOutput")

    with tile.TileContext(nc) as tc:
        with (
            tc.tile_pool(name="const", bufs=1) as cpool,
            tc.tile_pool(name="ld", bufs=3) as ldpool,
            tc.tile_pool(name="mid", bufs=3) as midpool,
            tc.tile_pool(name="unb", bufs=6) as unbpool,
            tc.tile_pool(name="bsd", bufs=4) as bsdpool,
            tc.tile_pool(name="msk", bufs=3) as mskpool,
            tc.tile_pool(name="sm", bufs=6) as smpool,
        ):
            # ---- constants (once) ----
            crow = cpool.tile([P, 64], f32, tag="crow")
            nc.sync.dma_start(crow[:1, :], cst[:])
            cbrow = cpool.tile([P, N_EXPERTS], f32, tag="cbrow")
            nc.sync.dma_start(cbrow[:1, :], cb[:])
            cball = cpool.tile([P, 64 + N_EXPERTS], f32, tag="cball")
            nc.gpsimd.partition_broadcast(cball[:, :64], crow[:1, :])
            nc.gpsimd.partition_broadcast(cball[:, 64:], cbrow[:1, :])
            marks_f = cball[:, 0:32]          # [P,32] f32 marks per (b, k)
            offs_f = cball[:, 32:64]          # [P,32] f32 offsets 256*b
            cb_rep = cball[:, 64:]            # [P,256] f32 bias
            # bf16 scatter payload + u16 offsets
            marks_bf = cpool.tile([P, 32], bf16, tag="marks_bf")
            nc.vector.tensor_copy(marks_bf[:], marks_f)
            offs_u16 = cpool.tile([P, 32], u16, tag="offs_u16")
            nc.vector.tensor_copy(offs_u16[:], offs_f)
            # cb replicated x4 along free for batched stt
            cb4 = cpool.tile([P, E4], f32, tag="cb4")
            for b in range(B):
                nc.vector.tensor_copy(cb4[:, b * N_EXPERTS:(b + 1) * N_EXPERTS], cb_rep)

            state = {}

            def tokslice(i):
                return slice(i * BTOK, (i + 1) * BTOK)

            def s_load(i):
                xt = ldpool.tile([P, E4], f32, tag="xt")
                nc.sync.dma_start(
                    xt[:].rearrange("p (b c) -> p b c", b=B),
                    x[tokslice(i), :].rearrange("(b p) c -> p b c", b=B),
                )
                state[("xt", i)] = xt

            def s_act(i):
                xt = state.pop(("xt", i))
                et = midpool.tile([P, E4], f32, tag="et")
                nc.scalar.activation(et[:], xt[:], Act.Exp, scale=-1.0)
                pt = midpool.tile([P, E4], f32, tag="pt")
                nc.scalar.activation(pt[:], et[:], Act.Identity, bias=1.0)
                state[("pt", i)] = pt

            def s_recip_bias(i):
                pt = state.pop(("pt", i))
                rt = unbpool.tile([P, E4], f32, tag="rt")
                nc.vector.reciprocal(rt[:], pt[:])          # s = sigmoid
                state[("rt", i)] = rt
                bt = bsdpool.tile([P, E4], f32, tag="bt")
                h = E4 // 2
                nc.gpsimd.tensor_tensor(bt[:, :h], rt[:, :h], cb4[:, :h], op=Alu.add)
                nc.gpsimd.tensor_tensor(bt[:, h:], rt[:, h:], cb4[:, h:], op=Alu.add)
                state[("bt", i)] = bt

            def s_groups(i):
                bt = state[("bt", i)]
                gval = smpool.tile([P, 64 * B], f32, tag="gval")
                for b in range(B):
                    for g in range(N_GROUP):
                        nc.vector.max(
                            gval[:, b * 64 + g * 8: b * 64 + (g + 1) * 8],
                            bt[:, b * N_EXPERTS + g * GROUP_SIZE:
                               b * N_EXPERTS + (g + 1) * GROUP_SIZE],
                        )
                # group scores = m1 + m2 (fl), batched
                gv = gval[:].rearrange("p (b g r) -> p b g r", b=B, g=N_GROUP)
                gs = smpool.tile([P, 8 * B], f32, tag="gs")
                gsv = gs[:].rearrange("p (b g) -> p b g", b=B)
                nc.vector.tensor_tensor(gsv, gv[:, :, :, 0], gv[:, :, :, 1], op=Alu.add)
                gst = smpool.tile([P, 8 * B], f32, tag="gst")
                for b in range(B):
                    nc.vector.max(gst[:, b * 8:(b + 1) * 8], gs[:, b * 8:(b + 1) * 8])
                # keep-mask = (gs >= 4th-largest)  [1.0 / 0.0]
                thr = (
                    gst[:].rearrange("p (b g) -> p b g", b=B)[:, :, 3:4]
                    .to_broadcast([P, B, N_GROUP])
                )
                km = smpool.tile([P, 8 * B], f32, tag="km")
                kmv = km[:].rearrange("p (b g) -> p b g", b=B)
                nc.vector.scalar_tensor_tensor(
                    kmv, gsv, 0.0, thr, op0=Alu.add, op1=Alu.is_ge
                )
                state[("gval", i)] = gval
                state[("km", i)] = km

            def s_mask(i):
                if not MASKED_SEARCH:
                    return
                bt = state[("bt", i)]
                km = state[("km", i)]
                kmv = km[:].rearrange("p (b g) -> p b g", b=B)
                mx = mskpool.tile([P, E4], f32, tag="mx")
                nc.gpsimd.tensor_tensor(
                    mx[:].rearrange("p (b g k) -> p b g k", b=B, g=N_GROUP),
                    bt[:].rearrange("p (b g k) -> p b g k", b=B, g=N_GROUP),
                    kmv.unsqueeze(3).to_broadcast([P, B, N_GROUP, GROUP_SIZE]),
                    op=Alu.mult,
                )                                            # masked = biased * keep
                state[("mx", i)] = mx

            def s_select(i):
                gval = state.pop(("gval", i))
                km = state.pop(("km", i))
                kmv = km[:].rearrange("p (b g) -> p b g", b=B)
                mx = state.pop(("mx", i)) if MASKED_SEARCH else state[("bt", i)]
                cand = smpool.tile([P, 64 * B], f32, tag="cand")
                nc.vector.scalar_tensor_tensor(
                    cand[:].rearrange("p (b g r) -> p b g r", b=B, g=N_GROUP),
                    gval[:].rearrange("p (b g r) -> p b g r", b=B, g=N_GROUP),
                    0.0,
                    kmv.unsqueeze(3).to_broadcast([P, B, N_GROUP, 8]),
                    op0=Alu.add,
                    op1=Alu.mult,
                )
                v8 = smpool.tile([P, 8 * B], f32, tag="v8")
                for b in range(B):
                    nc.vector.max(v8[:, b * 8:(b + 1) * 8], cand[:, b * 64:(b + 1) * 64])
                eidx = smpool.tile([P, 8 * B], u16, tag="eidx")
                for b in range(B):
                    nc.vector.max_index(
                        eidx[:, b * 8:(b + 1) * 8],
                        v8[:, b * 8:(b + 1) * 8],
                        mx[:, b * N_EXPERTS:(b + 1) * N_EXPERTS],
                    )
                state.pop(("bt", i))
                # scatter indices: eidx + 256*b, int16
                sidx = smpool.tile([P, 8 * B], u16, tag="sidx")
                nc.vector.tensor_tensor(sidx[:], eidx[:], offs_u16[:], op=Alu.add)
                mk = mskpool.tile([P, E4], bf16, tag="mk")
                nc.gpsimd.local_scatter(
                    mk[:], marks_bf[:], sidx[:].bitcast(i16),
                    channels=P, num_elems=E4, num_idxs=8 * B,
                )
                state[("eidx", i)] = eidx
                state[("mk", i)] = mk

            def s_weights(i):
                rt = state.pop(("rt", i))
                mk = state.pop(("mk", i))
                eidx = state.pop(("eidx", i))
                q = mskpool.tile([P, E4], f32, tag="q")
                h = E4 // 2
                nc.gpsimd.tensor_tensor(q[:, :h], rt[:, :h], mk[:, :h], op=Alu.add)
                nc.gpsimd.tensor_tensor(q[:, h:], rt[:, h:], mk[:, h:], op=Alu.add)
                u8m = smpool.tile([P, 8 * B], f32, tag="u8m")
                for b in range(B):
                    nc.vector.max(u8m[:, b * 8:(b + 1) * 8], q[:, b * N_EXPERTS:(b + 1) * N_EXPERTS])
                ut = smpool.tile([P, 8 * B], f32, tag="ut")
                nc.vector.tensor_tensor(ut[:], u8m[:], marks_f, op=Alu.subtract)
                den = smpool.tile([P, B], f32, tag="den")
                nc.vector.tensor_reduce(
                    den[:], ut[:].rearrange("p (b k) -> p b k", b=B), op=Alu.add, axis=X
                )
                rden = smpool.tile([P, B], f32, tag="rden")
                nc.vector.reciprocal(rden[:], den[:])
                wt = smpool.tile([P, 8 * B], f32, tag="wt")
                nc.vector.scalar_tensor_tensor(
                    wt[:].rearrange("p (b k) -> p b k", b=B),
                    ut[:].rearrange("p (b k) -> p b k", b=B),
                    2.5,
                    rden[:].unsqueeze(2).to_broadcast([P, B, 8]),
                    op0=Alu.mult,
                    op1=Alu.mult,
                )                                            # w = u*2.5/den
                nc.sync.dma_start(
                    ow[tokslice(i), :].rearrange("(b p) c -> p b c", b=B),
                    wt[:].rearrange("p (b k) -> p b k", b=B),
                )
                nc.sync.dma_start(
                    oi[tokslice(i), :].rearrange("(b p) c -> p b c", b=B),
                    eidx[:].rearrange("p (b k) -> p b k", b=B),
                )

            # software pipeline: oldest batch's latest stages first each round
            LAG_ACT, LAG_RB, LAG_GRP, LAG_MSK, LAG_SEL, LAG_W = 1, 2, 3, 4, 5, 7
            for i in range(n_batches + LAG_W):
                if LAG_SEL <= i < n_batches + LAG_SEL:
                    s_select(i - LAG_SEL)
                if LAG_W <= i < n_batches + LAG_W:
                    s_weights(i - LAG_W)
                if LAG_MSK <= i < n_batches + LAG_MSK:
                    s_mask(i - LAG_MSK)
                if LAG_GRP <= i < n_batches + LAG_GRP:
                    s_groups(i - LAG_GRP)
                if LAG_RB <= i < n_batches + LAG_RB:
                    s_recip_bias(i - LAG_RB)
                if LAG_ACT <= i < n_batches + LAG_ACT:
                    s_act(i - LAG_ACT)
                if i < n_batches:
                    s_load(i)

    nc.finalize()
    return nc


def get_module(tokens_per_core: int = TOK_PER_CORE):
    if tokens_per_core not in _COMPILED:
        _COMPILED[tokens_per_core] = _build(tokens_per_core)
    return _COMPILED[tokens_per_core]


def run(router_logits: np.ndarray, correction_bias: np.ndarray, trace: bool = False):
    """Shard across 8 cores, run, gather. Returns (idx, w[, perf])."""
    from concourse.bass_utils import run_bass_kernel_spmd

    n = router_logits.shape[0]
    tpc = n // N_CORES
    nc = get_module(tpc)
    cbm = np.ascontiguousarray(correction_bias.reshape(1, N_EXPERTS), dtype=np.float32)
    cstm = _consts_row()
    in_maps = [
        {
            "x": np.ascontiguousarray(
                router_logits[i * tpc:(i + 1) * tpc], dtype=np.float32
            ),
            "cb": cbm,
            "cst": cstm,
        }
        for i in range(N_CORES)
    ]
    res = run_bass_kernel_spmd(nc, in_maps, core_ids=list(range(N_CORES)), trace=trace)
    w = np.concatenate([r["ow"] for r in res.results], axis=0)
    idx = np.concatenate([r["oi"] for r in res.results], axis=0).astype(np.int32)
    if trace:
        return idx, w, res
    return idx, w


def kernel(router_logits: np.ndarray, correction_bias: np.ndarray):
    idx, w = run(np.asarray(router_logits), np.asarray(correction_bias))
    return idx.astype(np.int32), w.astype(np.float32)

